# revision 9
# baseline (speedup 1.0000x reference)
"""Trainium2 Bass kernel for nn_AttentionBlock (B=2, C=256, D=H=W=16).

Pipeline: GroupNorm(8) -> 1x1x1 conv QKV -> single-head attention over
N=4096 spatial tokens -> 1x1x1 conv proj -> residual.

Sharding: 8 cores = 2 batches x 4 query-chunks of 1024 tokens.  Each core
computes group-norm stats + K/V' for its full batch (redundantly across the
4 cores sharing a batch) and attention only for its 1024 queries.

Key algebraic folds (all exact in real arithmetic):
  - GroupNorm: xn = x*scale_c + shift_c with per-channel scale/shift derived
    from group stats; scale is folded into the QKV weights (W' = W * scale),
    shift into per-output-channel biases (W @ shift).
  - K bias drops entirely (adds a per-query constant to scores -> cancels in
    softmax).
  - proj is folded into V on the host: V' = (proj_w @ Wv) @ xn, so the
    attention output matmul directly produces the projected output.
  - 1/sqrt(C) score scale folded into Wq on the host.
  - softmax denominator via a ones-column appended to V'^T (column 256 of
    the out-matmul accumulates sum_k exp(s)).

Layouts: scores are computed transposed (keys on partitions, queries on the
free axis) so the probability tiles are directly usable as the stationary
operand of the output matmul; the output arrives as [query, channel] and is
transposed back 128x128 at the end via the PE transpose path.

Matmuls use float32r (full PE rate, ~TF32 precision, fp32 accumulate).
"""

import os
import sys

import numpy as np

if "/opt/trn_rl_repo" not in sys.path:
    sys.path.insert(0, "/opt/trn_rl_repo")

import concourse.bass as bass
import concourse.mybir as mybir
import concourse.tile as tile
from concourse.bass_utils import run_bass_kernel_spmd

F32 = mybir.dt.float32
F32R = mybir.dt.float32r
AF = mybir.ActivationFunctionType

B = 2
C = 256
N = 4096          # D*H*W tokens
NQ = 1024         # queries per core
G = 8             # groupnorm groups
GS = C // G       # 32 channels per group (== DVE transpose block size)
EPS = 1e-5
NCORES = 8

LAST_RESULT = None  # BassKernelResults of the most recent run (for profiling)
SPLIT = True  # apply split_waits (needed for walrus; CoreSim can't sim the nops)


_WS_CTR = [0]


def split_waits(nc, cap=1):
    """walrus (this build) allows a single sync wait per instruction; move
    excess sync_info.on_wait entries onto same-engine NoOps inserted before
    the instruction."""
    for fn in nc.m.functions:
        for blk in fn.blocks:
            out = []
            changed = False
            for ins in blk.instructions:
                si = ins.sync_info
                waits = list(si.on_wait) if si is not None else []
                if len(waits) > cap:
                    for i in range(0, len(waits) - cap, cap):
                        nop = mybir.InstNoOp(
                            name=f"I-waitsplit-{_WS_CTR[0]}",
                            engine=ins.engine,
                            ins=[], outs=[],
                        )
                        nop.sync_info = mybir.SyncInfo(
                            on_wait=waits[i:i + cap], on_update=[]
                        )
                        _WS_CTR[0] += 1
                        out.append(nop)
                    ins.sync_info = mybir.SyncInfo(
                        on_wait=waits[len(waits) - cap:],
                        on_update=list(si.on_update),
                    )
                    changed = True
                out.append(ins)
            if changed:
                blk.instructions = out


def build_bass():
    nc = bass.Bass(trn_type="TRN2")

    # ---- DRAM I/O ----
    xb_d = nc.dram_tensor("xb", [2, 128, N], F32R, kind="ExternalInput")
    xq_d = nc.dram_tensor("xq", [2, 128, NQ], F32R, kind="ExternalInput")
    wqk_d = nc.dram_tensor("wqk", [2, 128, 512], F32, kind="ExternalInput")
    wpv_d = nc.dram_tensor("wpv", [2, 128, 256], F32, kind="ExternalInput")
    # packed per-channel consts: [qb, cb, norm_w, norm_b]
    scb_d = nc.dram_tensor("scb", [2, 128, 4], F32, kind="ExternalInput")
    id_d = nc.dram_tensor("ident", [128, 128], F32, kind="ExternalInput")
    out_d = nc.dram_tensor("out", [2, 128, NQ], F32, kind="ExternalOutput")

    with tile.TileContext(nc) as tc:
        with (
            tc.tile_pool(name="consts", bufs=1) as consts,
            tc.tile_pool(name="work", bufs=3) as work,
            tc.tile_pool(name="small", bufs=4) as small,
            tc.tile_pool(name="ppool", bufs=2, space="PSUM") as ppool,
            tc.tile_pool(name="psS", bufs=2, space="PSUM") as psS,
            tc.tile_pool(name="psO", bufs=1, space="PSUM") as psO,
        ):
            # ---- persistent SBUF tiles ----
            ident = consts.tile([128, 128], F32, tag="ident")
            nc.sync.dma_start(out=ident, in_=id_d[:])

            wqk = []
            wpv = []
            scb = []
            xb = []
            xq = []
            for t in range(2):
                w1 = consts.tile([128, 512], F32, tag=f"wqk{t}")
                nc.sync.dma_start(out=w1, in_=wqk_d[t])
                wqk.append(w1)
                w2 = consts.tile([128, 256], F32, tag=f"wpv{t}")
                nc.sync.dma_start(out=w2, in_=wpv_d[t])
                wpv.append(w2)
                s = consts.tile([128, 4], F32, tag=f"scb{t}")
                nc.sync.dma_start(out=s, in_=scb_d[t])
                scb.append(s)
                xt = consts.tile([128, N], F32R, tag=f"xb{t}")
                for ch in range(4):
                    nc.sync.dma_start(
                        out=xt[:, ch * 1024:(ch + 1) * 1024],
                        in_=xb_d[t, :, ch * 1024:(ch + 1) * 1024],
                    )
                xb.append(xt)
                xqt = consts.tile([128, NQ], F32R, tag=f"xq{t}")
                nc.sync.dma_start(out=xqt, in_=xq_d[t])
                xq.append(xqt)

            # ---- group-norm stats -> per-channel scale/shift ----
            scale = []
            shift = []
            for t in range(2):
                xr = xb[t].bitcast(F32).rearrange("p (s c) -> p s c", c=512)
                st = work.tile([128, 8, 6], F32, tag="bnst")
                for i in range(8):
                    nc.vector.bn_stats(out=st[:, i, :], in_=xr[:, i, :])
                mv = small.tile([128, 2], F32, tag="mv")
                nc.vector.bn_aggr(out=mv, in_=st)

                # per-partition mean and E[x^2], broadcast along free axis
                e2 = small.tile([128, 1], F32, tag="e2")
                nc.vector.tensor_mul(e2, mv[:, 0:1], mv[:, 0:1])
                nc.vector.tensor_add(e2, e2, mv[:, 1:2])
                bm = work.tile([128, GS], F32, tag="bm")
                be = work.tile([128, GS], F32, tag="be")
                nc.vector.tensor_copy(bm, mv[:, 0:1].to_broadcast([128, GS]))
                nc.vector.tensor_copy(be, e2.to_broadcast([128, GS]))
                # 32x32 block transpose: row p of trm/tre now holds the 32
                # per-channel stats of p's own group -> free-axis reduce gives
                # group sums broadcast to every channel of the group.
                trm = work.tile([128, GS], F32, tag="trm")
                tre = work.tile([128, GS], F32, tag="tre")
                nc.vector.transpose(trm, bm)
                nc.vector.transpose(tre, be)
                mean_c = small.tile([128, 1], F32, tag="meanc")
                e2_c = small.tile([128, 1], F32, tag="e2c")
                nc.vector.reduce_sum(mean_c, trm, axis=mybir.AxisListType.X)
                nc.vector.reduce_sum(e2_c, tre, axis=mybir.AxisListType.X)
                inv = 1.0 / (GS * N)
                nc.vector.tensor_scalar_mul(mean_c, mean_c, inv * N)
                nc.vector.tensor_scalar_mul(e2_c, e2_c, inv * N)
                # var = E[x^2] - mean^2 (+eps)
                ve = small.tile([128, 1], F32, tag="ve")
                nc.vector.tensor_mul(ve, mean_c, mean_c)
                nc.vector.tensor_sub(ve, e2_c, ve)
                nc.vector.tensor_scalar_add(ve, ve, EPS)
                # rstd = 1/sqrt(ve), ACT sqrt + exact reciprocal + 1 Newton step
                r0 = small.tile([128, 1], F32, tag="r0")
                nc.scalar.activation(out=r0, in_=ve, func=AF.Sqrt)
                nc.vector.reciprocal(r0, r0)
                t2 = small.tile([128, 1], F32, tag="t2")
                nc.vector.tensor_mul(t2, r0, r0)
                nc.vector.tensor_mul(t2, t2, ve)
                nc.vector.tensor_scalar(
                    t2, t2, -0.5, 1.5, mybir.AluOpType.mult, mybir.AluOpType.add
                )
                rstd = small.tile([128, 1], F32, tag="rstd")
                nc.vector.tensor_mul(rstd, r0, t2)
                # scale = rstd * norm_w ; shift = norm_b - mean*scale
                sc = consts.tile([128, 1], F32, tag=f"scale{t}")
                nc.vector.tensor_mul(sc, rstd, scb[t][:, 2:3])
                sh = consts.tile([128, 1], F32, tag=f"shift{t}")
                nc.vector.tensor_mul(sh, mean_c, sc)
                nc.vector.tensor_sub(sh, scb[t][:, 3:4], sh)
                scale.append(sc)
                shift.append(sh)

            # ---- fold scale into weights ----
            wqk_s = []
            wpv_s = []
            for t in range(2):
                ws = consts.tile([128, 512], F32R, tag=f"wqks{t}")
                nc.vector.tensor_scalar_mul(ws, wqk[t], scale[t])
                wqk_s.append(ws)
                ws2 = consts.tile([128, 256], F32R, tag=f"wpvs{t}")
                nc.vector.tensor_scalar_mul(ws2, wpv[t], scale[t])
                wpv_s.append(ws2)

            # ---- shift-induced biases: qbias = Wq @ shift + qb ; fbias = Wpv @ shift + cb
            qbias = []
            fbias = []
            for m in range(2):
                ps = ppool.tile([128, 512], F32, tag="pmisc")
                for t in range(2):
                    nc.tensor.matmul(
                        ps[:, 0:1],
                        lhsT=wqk[t][:, m * 128:(m + 1) * 128],
                        rhs=shift[t],
                        start=(t == 0),
                        stop=(t == 1),
                    )
                qb_m = consts.tile([128, 1], F32, tag=f"qbias{m}")
                nc.vector.tensor_add(qb_m, ps[:, 0:1], scb[m][:, 0:1])
                qbias.append(qb_m)
            for m in range(2):
                ps = ppool.tile([128, 512], F32, tag="pmisc")
                for t in range(2):
                    nc.tensor.matmul(
                        ps[:, 0:1],
                        lhsT=wpv[t][:, m * 128:(m + 1) * 128],
                        rhs=shift[t],
                        start=(t == 0),
                        stop=(t == 1),
                    )
                fb_m = consts.tile([128, 1], F32, tag=f"fbias{m}")
                nc.vector.tensor_add(fb_m, ps[:, 0:1], scb[m][:, 1:2])
                fbias.append(fb_m)

            # ---- QKV matmuls (float32r) ----
            Q = [consts.tile([128, NQ], F32R, tag=f"Q{m}", name=f"Q{m}") for m in range(2)]
            for m in range(2):
                for ch in range(NQ // 512):
                    ps = ppool.tile([128, 512], F32, tag="pmisc")
                    for t in range(2):
                        nc.tensor.matmul(
                            ps,
                            lhsT=wqk_s[t][:, m * 128:(m + 1) * 128],
                            rhs=xq[t][:, ch * 512:(ch + 1) * 512],
                            start=(t == 0),
                            stop=(t == 1),
                        )
                    nc.vector.tensor_scalar_add(
                        Q[m][:, ch * 512:(ch + 1) * 512], ps, qbias[m]
                    )

            K = [consts.tile([128, N], F32R, tag=f"K{m}", name=f"K{m}") for m in range(2)]
            for m in range(2):
                for ch in range(N // 512):
                    ps = ppool.tile([128, 512], F32, tag="pmisc")
                    for t in range(2):
                        nc.tensor.matmul(
                            ps,
                            lhsT=wqk_s[t][:, (2 + m) * 128:(3 + m) * 128],
                            rhs=xb[t][:, ch * 512:(ch + 1) * 512],
                            start=(t == 0),
                            stop=(t == 1),
                        )
                    nc.scalar.activation(
                        out=K[m][:, ch * 512:(ch + 1) * 512], in_=ps, func=AF.Copy
                    )

            # V'^T tiles: [token_tile j][128 tokens, 256 ch + ones column]
            # 258-wide: f32r moving operand needs an even free dim.
            # col 256 = ones (softmax denominator), col 257 = zeros (pad).
            VT = consts.tile([128, 32, 258], F32R, tag="VT")
            ones_t = consts.tile([128, 2], F32, tag="ones")
            nc.vector.memset(ones_t[:, 0:1], 1.0)
            nc.vector.memset(ones_t[:, 1:2], 0.0)
            nc.vector.tensor_copy(
                VT[:, :, 256:258], ones_t[:, None, :].to_broadcast([128, 32, 2])
            )
            for j in range(32):
                ps = ppool.tile([128, 512], F32, tag="pmisc")
                for t in range(2):
                    nc.tensor.matmul(
                        ps[:, 0:256],
                        lhsT=xb[t][:, j * 128:(j + 1) * 128],
                        rhs=wpv_s[t],
                        start=(t == 0),
                        stop=(t == 1),
                    )
                nc.vector.tensor_copy(VT[:, j, 0:256], ps[:, 0:256])

            # ---- attention ----
            fin = [consts.tile([128, NQ], F32, tag=f"fin{m}", name=f"fin{m}") for m in range(2)]
            for qt in range(NQ // 512):
                po = [psO.tile([128, 258], F32, tag=f"po{qs}", name=f"po{qt}_{qs}") for qs in range(4)]
                for j in range(32):
                    ss = psS.tile([128, 512], F32, tag="ss")
                    for t in range(2):
                        nc.tensor.matmul(
                            ss,
                            lhsT=K[t][:, j * 128:(j + 1) * 128],
                            rhs=Q[t][:, qt * 512:(qt + 1) * 512],
                            start=(t == 0),
                            stop=(t == 1),
                        )
                    pe = work.tile([128, 512], F32R, tag="pexp")
                    nc.scalar.activation(out=pe, in_=ss, func=AF.Exp)
                    for qs in range(4):
                        nc.tensor.matmul(
                            po[qs],
                            lhsT=pe[:, qs * 128:(qs + 1) * 128],
                            rhs=VT[:, j, :],
                            start=(j == 0),
                            stop=(j == 31),
                        )
                # normalize + transpose back to [channel, query]
                for qs in range(4):
                    zr = small.tile([128, 1], F32, tag="zr")
                    nc.vector.reciprocal(zr, po[qs][:, 256:257])
                    ao = work.tile([128, 256], F32, tag="ao")
                    nc.vector.tensor_scalar_mul(ao, po[qs][:, 0:256], zr)
                    col = (qt * 4 + qs) * 128
                    for m in range(2):
                        tp = ppool.tile([128, 512], F32, tag="pmisc")
                        nc.tensor.transpose(
                            tp[:, 0:128], ao[:, m * 128:(m + 1) * 128], ident
                        )
                        nc.scalar.activation(
                            out=fin[m][:, col:col + 128],
                            in_=tp[:, 0:128],
                            func=AF.Identity,
                            bias=fbias[m],
                        )

            # ---- residual + store ----
            for m in range(2):
                nc.vector.tensor_add(fin[m], fin[m], xq[m].bitcast(F32))
                nc.sync.dma_start(out=out_d[m], in_=fin[m])

    if SPLIT:
        split_waits(nc)
    return nc


_CACHED = None
_RUNNER = None


def _get_nc():
    global _CACHED
    if _CACHED is None:
        _CACHED = build_bass()
    return _CACHED


def _get_runner():
    """Cached jitted shard_map runner over 8 cores (mirrors
    bass2jax.run_bass_via_pjrt, minus donation, so the compiled executable
    and device-resident inputs can be reused across calls)."""
    global _RUNNER
    if _RUNNER is not None:
        return _RUNNER
    import jax
    from jax.experimental.shard_map import shard_map
    from jax.sharding import Mesh, PartitionSpec
    from concourse import bass2jax, mybir as mb
    from concourse.bass2jax import _bass_exec_p, install_neuronx_cc_hook

    nc = _get_nc()
    install_neuronx_cc_hook()
    assert nc.dbg_addr is None
    partition_name = nc.partition_id_tensor.name if nc.partition_id_tensor else None

    in_names = []
    out_names = []
    out_avals = []
    zero_outs = []
    for alloc in nc.m.functions[0].allocations:
        if not isinstance(alloc, mb.MemoryLocationSet):
            continue
        name = alloc.memorylocations[0].name
        if alloc.kind == "ExternalInput":
            if name != partition_name:
                in_names.append(name)
        elif alloc.kind == "ExternalOutput":
            out_names.append(name)
            shape = tuple(alloc.tensor_shape)
            dtype = mb.dt.np(alloc.dtype)
            out_avals.append(jax.core.ShapedArray(shape, dtype))
            zero_outs.append(np.zeros(shape, dtype))
    n_params = len(in_names)
    all_in_names = in_names + out_names
    if partition_name is not None:
        all_in_names = all_in_names + [partition_name]

    def _body(*args):
        operands = list(args)
        if partition_name is not None:
            operands.append(bass2jax.partition_id_tensor())
        outs = _bass_exec_p.bind(
            *operands,
            out_avals=tuple(out_avals),
            in_names=tuple(all_in_names),
            out_names=tuple(out_names),
            lowering_input_output_aliases=(),
            sim_require_finite=True,
            sim_require_nnan=True,
            nc=nc,
        )
        return tuple(outs)

    devices = jax.devices()[:NCORES]
    mesh = Mesh(np.asarray(devices), ("core",))
    n_outs = len(out_names)
    sharded = jax.jit(
        shard_map(
            _body,
            mesh=mesh,
            in_specs=(PartitionSpec("core"),) * (n_params + n_outs),
            out_specs=(PartitionSpec("core"),) * n_outs,
            check_rep=False,
        ),
        keep_unused=True,
    )
    _RUNNER = (sharded, in_names, out_names, out_avals, zero_outs, mesh)
    return _RUNNER


def _concat_inputs(in_maps, in_names, zero_outs):
    concat_in = [
        np.concatenate([np.asarray(in_maps[c][name]) for c in range(NCORES)], axis=0)
        for name in in_names
    ]
    concat_zeros = [
        np.zeros((NCORES * z.shape[0], *z.shape[1:]), z.dtype) for z in zero_outs
    ]
    return concat_in, concat_zeros


def _run(in_maps):
    sharded, in_names, out_names, out_avals, zero_outs, mesh = _get_runner()
    concat_in, concat_zeros = _concat_inputs(in_maps, in_names, zero_outs)
    out_arrs = sharded(*concat_in, *concat_zeros)
    return [
        {
            name: np.asarray(out_arrs[i]).reshape(NCORES, *out_avals[i].shape)[c]
            for i, name in enumerate(out_names)
        }
        for c in range(NCORES)
    ]


def _host_prep(x, norm_w, norm_b, qkv_w, qkv_b, proj_w, proj_b):
    inv_sqrt_c = 1.0 / np.sqrt(np.float32(C)).astype(np.float32)
    wq = qkv_w[0:C] * inv_sqrt_c
    wk = qkv_w[C:2 * C]
    wv = qkv_w[2 * C:3 * C]
    wqkT = np.ascontiguousarray(
        np.concatenate([wq, wk], axis=0).T.reshape(C, 512)
    ).reshape(2, 128, 512)
    wpvT = np.ascontiguousarray((proj_w @ wv).T).reshape(2, 128, 256)
    qb = (qkv_b[0:C] * inv_sqrt_c).reshape(2, 128, 1)
    cb = (proj_w @ qkv_b[2 * C:3 * C] + proj_b).reshape(2, 128, 1)
    scb = np.ascontiguousarray(
        np.concatenate(
            [qb, cb, norm_w.reshape(2, 128, 1), norm_b.reshape(2, 128, 1)], axis=2
        )
    ).astype(np.float32)
    ident = np.eye(128, dtype=np.float32)

    xf = x.reshape(B, 2, 128, N)
    in_maps = []
    for core in range(NCORES):
        b, qi = divmod(core, NCORES // B)
        in_maps.append(
            {
                "xb": xf[b],
                "xq": np.ascontiguousarray(xf[b][:, :, qi * NQ:(qi + 1) * NQ]),
                "wqk": wqkT,
                "wpv": wpvT,
                "scb": scb,
                "ident": ident,
            }
        )
    return in_maps


def kernel(x, norm_w, norm_b, qkv_w, qkv_b, proj_w, proj_b):
    x = np.ascontiguousarray(np.asarray(x, dtype=np.float32))
    norm_w = np.asarray(norm_w, dtype=np.float32)
    norm_b = np.asarray(norm_b, dtype=np.float32)
    qkv_w = np.asarray(qkv_w, dtype=np.float32)
    qkv_b = np.asarray(qkv_b, dtype=np.float32)
    proj_w = np.asarray(proj_w, dtype=np.float32)
    proj_b = np.asarray(proj_b, dtype=np.float32)

    Bs, Cs = x.shape[0], x.shape[1]
    assert (Bs, Cs) == (B, C) and x.shape[2] * x.shape[3] * x.shape[4] == N

    in_maps = _host_prep(x, norm_w, norm_b, qkv_w, qkv_b, proj_w, proj_b)
    results = _run(in_maps)

    y = np.empty((B, C, N), dtype=np.float32)
    for core in range(NCORES):
        b, qi = divmod(core, NCORES // B)
        y[b, :, qi * NQ:(qi + 1) * NQ] = results[core]["out"].reshape(C, NQ)
    return y.reshape(x.shape)


def bench(in_maps, iters=50, warmup=3):
    """Amortized per-execution device time: device-resident inputs, back-to-
    back async executes, block at the end."""
    import time
    import jax
    from jax.sharding import NamedSharding, PartitionSpec

    sharded, in_names, out_names, out_avals, zero_outs, mesh = _get_runner()
    concat_in, concat_zeros = _concat_inputs(in_maps, in_names, zero_outs)
    sh = NamedSharding(mesh, PartitionSpec("core"))
    dev_in = [jax.device_put(a, sh) for a in concat_in]
    dev_zero = [jax.device_put(a, sh) for a in concat_zeros]
    for _ in range(warmup):
        out = sharded(*dev_in, *dev_zero)
    jax.block_until_ready(out)
    t0 = time.perf_counter()
    for _ in range(iters):
        out = sharded(*dev_in, *dev_zero)
    jax.block_until_ready(out)
    t1 = time.perf_counter()
    return (t1 - t0) / iters
